# revision 19
# baseline (speedup 1.0000x reference)
"""Bidirectional Chamfer loss on 8 Trainium2 NeuronCores.

Math: for each batch pair (p, q):
    D[i, j] = ||p_i||^2 + ||q_j||^2 - 2 p_i . q_j
    cd = mean_i min_j D[i, j] + mean_j min_i D[i, j]
    loss = 0.7 * mean_b cd_filtered + 0.3 * mean_b cd_nonfiltered

Mapping ("orientation B": gt points on PSUM partitions, pred rows on free):
  - Host packs, per (config, batch), K=24 bf16 matmul operands so one PE
    matmul emits D tiles directly into PSUM. fp32 values are split 3-way
    into bf16 (hi, mid, lo; x = x0+x1+x2 with |x_i| <= 2^-9|x_{i-1}|) and
    products keep the 6 dominant cross terms -> error ~2^-27 per term
    (hardware fp32/fp32r matmul paths are NOT full fp32 precision).
        per coord c:   G rows q_c{0,1,2}, P rows (-2 p_c){0,1,2},
                       pairs (0,0),(0,1),(1,0),(0,2),(1,1),(2,0)
        norm rows:     G [qq0,qq1,qq2,1,1,1] x P [1,1,1,pp0,pp1,pp2]
    -> psum[p, f] = D[pred row f, gt pt jt*128+p]
  - pred rows sharded 8 ways (512/1024 rows per core); gt replicated.
  - Per gt chunk jt (standard BIR ops only — custom DVE ops like
    tensor_tensor_reduce crash this runtime):
      DVE tensor_reduce(min) over pred rows -> EXACT fp32 per-gt-point min
          over this core's pred rows (host min-combines across cores)
      ACT scalar.copy downcasts the PSUM tile to a bf16 SBUF copy (parallel
          engine; PSUM has separate DVE and ACT read ports)
      DVE tensor_tensor(min) bf16 @2x: predacc accumulates pred-side
          partial mins across gt chunks into [128, R_core].
  - Epilogue: PE-transpose predacc in [128,128] chunks, free-axis reduce
    -> pred-side min per pred row (bf16-rounded, unbiased).
  - Host: tiny cross-core min/mean combine.
"""

import numpy as np

B = 4
NF = 4096
NN = 8192
NCORES = 8
RF = NF // NCORES   # 512 pred rows per core (filtered)
RN = NN // NCORES   # 1024 pred rows per core (nonfiltered)
BIG = 3.0e38
K24 = 24            # contraction rows of the split-bf16 matmul

# output column layout
N_M1 = B * (RF // 128 + RN // 128)          # 4*(4+8) = 48   pred-side mins
N_M2 = B * (NF // 128 + NN // 128)          # 4*(32+64) = 384 gt-side mins

_CACHE = {}


def build_nc():
    """Build the per-core Bass program (SPMD: same program, different data)."""
    from contextlib import ExitStack

    import concourse.mybir as mybir
    import concourse.tile as tile
    from concourse import bacc
    from concourse.masks import make_identity

    f32 = mybir.dt.float32
    bf16 = mybir.dt.bfloat16
    Alu = mybir.AluOpType

    nc = bacc.Bacc("TRN2", target_bir_lowering=False, debug=False)

    # pred (sharded), gt (replicated) operands, [B, K24, n]
    Pf = nc.dram_tensor("pf", [B, K24, RF], bf16, kind="ExternalInput").ap()
    Gf = nc.dram_tensor("gf", [B, K24, NF], bf16, kind="ExternalInput").ap()
    Pn = nc.dram_tensor("pn", [B, K24, RN], bf16, kind="ExternalInput").ap()
    Gn = nc.dram_tensor("gn", [B, K24, NN], bf16, kind="ExternalInput").ap()
    O1 = nc.dram_tensor("m1", [128, N_M1], f32, kind="ExternalOutput").ap()
    O2 = nc.dram_tensor("m2", [128, N_M2], f32, kind="ExternalOutput").ap()

    with tile.TileContext(nc) as tc, ExitStack() as ctx:
        const_pool = ctx.enter_context(tc.tile_pool(name="const", bufs=1))
        gpool = ctx.enter_context(tc.tile_pool(name="gt", bufs=2))
        ppool = ctx.enter_context(tc.tile_pool(name="pred", bufs=2))
        cpool = ctx.enter_context(tc.tile_pool(name="copy", bufs=4))
        apool = ctx.enter_context(tc.tile_pool(name="pacc", bufs=2))
        out_pool = ctx.enter_context(tc.tile_pool(name="outs", bufs=1))
        psum_pool = ctx.enter_context(tc.tile_pool(name="psum", bufs=3, space="PSUM"))
        epi_pool = ctx.enter_context(tc.tile_pool(name="epi", bufs=1, space="PSUM"))

        ident = const_pool.tile([128, 128], bf16)
        make_identity(nc, ident)
        m1t = out_pool.tile([128, N_M1], f32)
        m2t = out_pool.tile([128, N_M2], f32)

        m1col = 0
        m2col = 0
        for Pt, Gt, Npts, Rrows in ((Pf, Gf, NF, RF), (Pn, Gn, NN, RN)):
            n_jt = Npts // 128      # gt chunks
            n_mm = Rrows // 512     # matmuls (N=512) per gt chunk
            for b in range(B):
                sG = gpool.tile([K24, Npts], bf16, tag="gt")
                nc.sync.dma_start(sG[:], Gt[b])
                sP = ppool.tile([K24, Rrows], bf16, tag="pred")
                nc.sync.dma_start(sP[:], Pt[b])
                pacc = apool.tile([128, Rrows], bf16, tag="pacc")

                for jt in range(n_jt):
                    lhsT = sG[:, jt * 128 : (jt + 1) * 128]
                    ps = psum_pool.tile([128, Rrows], f32, tag="ps")
                    for h in range(n_mm):
                        nc.tensor.matmul(
                            ps[:, h * 512 : (h + 1) * 512],
                            lhsT=lhsT,
                            rhs=sP[:, h * 512 : (h + 1) * 512],
                            start=True,
                            stop=True,
                        )
                    nc.vector.tensor_reduce(
                        out=m2t[:, m2col + jt : m2col + jt + 1],
                        in_=ps[:],
                        axis=mybir.AxisListType.X,
                        op=Alu.min,
                    )
                    cp = cpool.tile([128, Rrows], bf16, tag="copy")
                    nc.scalar.copy(cp[:], ps[:])
                    if jt == 0:
                        nc.vector.tensor_copy(pacc[:], cp[:])
                    else:
                        nc.vector.tensor_tensor(
                            out=pacc[:], in0=cp[:], in1=pacc[:], op=Alu.min
                        )
                m2col += n_jt

                # epilogue: pred-side mins (partition-min of pacc via transpose)
                n_ch = Rrows // 128
                ep = epi_pool.tile([128, Rrows], bf16, tag="epi")
                for c in range(n_ch):
                    nc.tensor.transpose(
                        ep[:, c * 128 : (c + 1) * 128],
                        pacc[:, c * 128 : (c + 1) * 128],
                        ident,
                    )
                nc.vector.tensor_reduce(
                    out=m1t[:, m1col : m1col + n_ch],
                    in_=ep[:].rearrange("p (c k) -> p c k", k=128),
                    axis=mybir.AxisListType.X,
                    op=Alu.min,
                )
                m1col += n_ch

        nc.sync.dma_start(O1[:], m1t[:])
        nc.sync.dma_start(O2[:], m2t[:])

    # Bacc legalization: splits >1-wait instructions into EventSemaphore
    # chains (TRN2 allows 1 wait/inst), moves matmul waits, fuses nops.
    nc.compile()
    return nc


def _split3(x):
    """fp32 -> three bf16 arrays with x ~= b0+b1+b2 (error ~2^-27 |x|)."""
    import ml_dtypes

    bf = ml_dtypes.bfloat16
    b0 = x.astype(bf)
    r1 = (x - b0.astype(np.float32)).astype(np.float32)
    b1 = r1.astype(bf)
    r2 = (r1 - b1.astype(np.float32)).astype(np.float32)
    b2 = r2.astype(bf)
    return b0, b1, b2


# product-pair pattern per coordinate: (gt split idx, pred split idx)
_PAIRS = ((0, 0), (0, 1), (1, 0), (0, 2), (1, 1), (2, 0))


def pack_inputs(pred_filtered, gt_filtered, pred_nonfiltered, gt_nonfiltered):
    """Build per-core input maps (bf16 split operands)."""
    import ml_dtypes

    bf = ml_dtypes.bfloat16

    def mk(p, q):
        # returns (P [B,24,Np] , G [B,24,Nq]) bf16
        p = p.astype(np.float32)
        q = q.astype(np.float32)
        Bn, Np_, _ = p.shape
        Nq = q.shape[1]
        P = np.zeros((Bn, K24, Np_), bf)
        G = np.zeros((Bn, K24, Nq), bf)
        pp = np.sum(p * p, axis=-1, dtype=np.float32)
        qq = np.sum(q * q, axis=-1, dtype=np.float32)
        for c in range(3):
            ws = _split3(-2.0 * p[..., c])     # pred-side coord splits
            gs = _split3(q[..., c])            # gt-side coord splits
            for t, (gi, wi) in enumerate(_PAIRS):
                G[:, 6 * c + t, :] = gs[gi]
                P[:, 6 * c + t, :] = ws[wi]
        qqs = _split3(qq)
        pps = _split3(pp)
        for t in range(3):
            G[:, 18 + t, :] = qqs[t]
            P[:, 18 + t, :] = np.ones_like(pp, dtype=bf)
            G[:, 21 + t, :] = np.ones_like(qq, dtype=bf)
            P[:, 21 + t, :] = pps[t]
        return P, G

    pf_all, gf = mk(pred_filtered, gt_filtered)
    pn_all, gn = mk(pred_nonfiltered, gt_nonfiltered)
    gf = np.ascontiguousarray(gf)
    gn = np.ascontiguousarray(gn)

    in_maps = []
    for k in range(NCORES):
        in_maps.append(
            {
                "pf": np.ascontiguousarray(pf_all[:, :, k * RF : (k + 1) * RF]),
                "gf": gf,
                "pn": np.ascontiguousarray(pn_all[:, :, k * RN : (k + 1) * RN]),
                "gn": gn,
            }
        )
    return in_maps


def combine_outputs(results):
    """results: list (per core) of {"m1": [128,48], "m2": [128,384]} -> loss."""
    cds = {}
    for cfg, (Npts, Rrows, m1off, m2off) in (
        ("f", (NF, RF, 0, 0)),
        ("n", (NN, RN, B * (RF // 128), B * (NF // 128))),
    ):
        n_ch = Rrows // 128   # m1 cols per batch (pred rows / 128, per core)
        n_jt = Npts // 128    # m2 cols per batch (gt chunks)
        # pred-side: values are per-pred-row mins already; mean over all
        m1 = np.stack(
            [r["m1"][:, m1off : m1off + B * n_ch] for r in results], axis=0
        ).reshape(NCORES, 128, B, n_ch)
        pred_mean = m1.mean(axis=(0, 1, 3))  # [B]
        # gt-side: per-core partial mins -> min across cores, mean over gt
        m2 = np.stack(
            [r["m2"][:, m2off : m2off + B * n_jt] for r in results], axis=0
        )
        m2 = m2.min(axis=0).reshape(128, B, n_jt)
        gt_mean = m2.mean(axis=(0, 2))  # [B]
        cds[cfg] = (pred_mean + gt_mean).mean()
    return np.float32(0.7 * cds["f"] + 0.3 * cds["n"])


def kernel(pred_filtered, gt_filtered, pred_nonfiltered, gt_nonfiltered):
    from concourse.bass_utils import run_bass_kernel_spmd

    if "nc" not in _CACHE:
        _CACHE["nc"] = build_nc()
    in_maps = pack_inputs(
        pred_filtered, gt_filtered, pred_nonfiltered, gt_nonfiltered
    )
    res = run_bass_kernel_spmd(_CACHE["nc"], in_maps, core_ids=list(range(NCORES)))
    return combine_outputs(res.results)


# revision 22
# speedup vs baseline: 1.1444x; 1.1444x over previous
"""Bidirectional Chamfer loss on 8 Trainium2 NeuronCores.

Math: for each batch pair (p, q):
    D[i, j] = ||p_i||^2 + ||q_j||^2 - 2 p_i . q_j
    cd = mean_i min_j D[i, j] + mean_j min_i D[i, j]
    loss = 0.7 * mean_b cd_filtered + 0.3 * mean_b cd_nonfiltered

Mapping ("orientation B": gt points on PSUM partitions, pred rows on free):
  - Host packs, per (config, batch), K=24 bf16 matmul operands so one PE
    matmul emits D tiles directly into PSUM. fp32 values are split 3-way
    into bf16 (hi, mid, lo; x = x0+x1+x2 with |x_i| <= 2^-9|x_{i-1}|) and
    products keep the 6 dominant cross terms -> error ~2^-27 per term
    (hardware fp32/fp32r matmul paths are NOT full fp32 precision).
        per coord c:   G rows q_c{0,1,2}, P rows (-2 p_c){0,1,2},
                       pairs (0,0),(0,1),(1,0),(0,2),(1,1),(2,0)
        norm rows:     G [qq0,qq1,qq2,1,1,1] x P [1,1,1,pp0,pp1,pp2]
    -> psum[p, f] = D[pred row f, gt pt jt*128+p]
  - pred rows sharded 8 ways (512/1024 rows per core); gt replicated.
  - Per gt chunk jt (standard BIR ops only — custom DVE ops like
    tensor_tensor_reduce crash this runtime):
      ACT scalar.copy downcasts the PSUM tile to a bf16 SBUF copy (parallel
          engine; only ACT reads PSUM)
      DVE folding tree on the bf16 copy (pairwise tensor_tensor(min) @2x,
          then a short 1x reduce) -> per-gt-point min over this core's pred
          rows (host min-combines across cores)
      DVE tensor_tensor(min) bf16 @2x: predacc accumulates pred-side
          partial mins across gt chunks into [128, R_core].
  - Epilogue: PE-transpose predacc in [128,128] chunks, free-axis reduce
    -> pred-side min per pred row (bf16-rounded, unbiased).
  - Host: tiny cross-core min/mean combine.
"""

import numpy as np

B = 4
NF = 4096
NN = 8192
NCORES = 8
RF = NF // NCORES   # 512 pred rows per core (filtered)
RN = NN // NCORES   # 1024 pred rows per core (nonfiltered)
BIG = 3.0e38
K24 = 24            # contraction rows of the split-bf16 matmul

# output column layout
N_M1 = B * (RF // 128 + RN // 128)          # 4*(4+8) = 48   pred-side mins
N_M2 = B * (NF // 128 + NN // 128)          # 4*(32+64) = 384 gt-side mins

_CACHE = {}


def build_nc():
    """Build the per-core Bass program (SPMD: same program, different data)."""
    from contextlib import ExitStack

    import concourse.mybir as mybir
    import concourse.tile as tile
    from concourse import bacc
    from concourse.masks import make_identity

    f32 = mybir.dt.float32
    bf16 = mybir.dt.bfloat16
    Alu = mybir.AluOpType

    nc = bacc.Bacc("TRN2", target_bir_lowering=False, debug=False)

    # pred (sharded), gt (replicated) operands, [B, K24, n]
    Pf = nc.dram_tensor("pf", [B, K24, RF], bf16, kind="ExternalInput").ap()
    Gf = nc.dram_tensor("gf", [B, K24, NF], bf16, kind="ExternalInput").ap()
    Pn = nc.dram_tensor("pn", [B, K24, RN], bf16, kind="ExternalInput").ap()
    Gn = nc.dram_tensor("gn", [B, K24, NN], bf16, kind="ExternalInput").ap()
    O1 = nc.dram_tensor("m1", [128, N_M1], f32, kind="ExternalOutput").ap()
    O2 = nc.dram_tensor("m2", [128, N_M2], f32, kind="ExternalOutput").ap()

    with tile.TileContext(nc) as tc, ExitStack() as ctx:
        const_pool = ctx.enter_context(tc.tile_pool(name="const", bufs=1))
        gpool = ctx.enter_context(tc.tile_pool(name="gt", bufs=2))
        ppool = ctx.enter_context(tc.tile_pool(name="pred", bufs=2))
        cpool = ctx.enter_context(tc.tile_pool(name="copy", bufs=4))
        spool = ctx.enter_context(tc.tile_pool(name="fold", bufs=3))
        apool = ctx.enter_context(tc.tile_pool(name="pacc", bufs=2))
        out_pool = ctx.enter_context(tc.tile_pool(name="outs", bufs=1))
        psum_pool = ctx.enter_context(tc.tile_pool(name="psum", bufs=3, space="PSUM"))
        epi_pool = ctx.enter_context(tc.tile_pool(name="epi", bufs=1, space="PSUM"))

        ident = const_pool.tile([128, 128], bf16)
        make_identity(nc, ident)
        m1t = out_pool.tile([128, N_M1], f32)
        m2t = out_pool.tile([128, N_M2], f32)

        m1col = 0
        m2col = 0
        for Pt, Gt, Npts, Rrows in ((Pf, Gf, NF, RF), (Pn, Gn, NN, RN)):
            n_jt = Npts // 128      # gt chunks
            n_mm = Rrows // 512     # matmuls (N=512) per gt chunk
            for b in range(B):
                sG = gpool.tile([K24, Npts], bf16, tag="gt")
                nc.sync.dma_start(sG[:], Gt[b])
                sP = ppool.tile([K24, Rrows], bf16, tag="pred")
                nc.sync.dma_start(sP[:], Pt[b])
                pacc = apool.tile([128, Rrows], bf16, tag="pacc")

                for jt in range(n_jt):
                    lhsT = sG[:, jt * 128 : (jt + 1) * 128]
                    ps = psum_pool.tile([128, Rrows], f32, tag="ps")
                    for h in range(n_mm):
                        nc.tensor.matmul(
                            ps[:, h * 512 : (h + 1) * 512],
                            lhsT=lhsT,
                            rhs=sP[:, h * 512 : (h + 1) * 512],
                            start=True,
                            stop=True,
                        )
                    cp = cpool.tile([128, Rrows], bf16, tag="copy")
                    nc.scalar.copy(cp[:], ps[:])
                    # m2: folding tree min over pred rows (bf16 @2x)
                    sc = spool.tile([128, Rrows // 2], bf16, tag="fold")
                    w = Rrows // 2
                    nc.vector.tensor_tensor(
                        out=sc[:, :w], in0=cp[:, :w], in1=cp[:, w:], op=Alu.min
                    )
                    while w > 128:
                        h = w // 2
                        nc.vector.tensor_tensor(
                            out=sc[:, :h], in0=sc[:, :h], in1=sc[:, h:w],
                            op=Alu.min,
                        )
                        w = h
                    nc.vector.tensor_reduce(
                        out=m2t[:, m2col + jt : m2col + jt + 1],
                        in_=sc[:, :w],
                        axis=mybir.AxisListType.X,
                        op=Alu.min,
                    )
                    # m1 partial: accumulate pred-side mins across gt chunks
                    if jt == 0:
                        nc.vector.tensor_copy(pacc[:], cp[:])
                    else:
                        nc.vector.tensor_tensor(
                            out=pacc[:], in0=cp[:], in1=pacc[:], op=Alu.min
                        )
                m2col += n_jt

                # epilogue: pred-side mins (partition-min of pacc via transpose)
                n_ch = Rrows // 128
                ep = epi_pool.tile([128, Rrows], bf16, tag="epi")
                for c in range(n_ch):
                    nc.tensor.transpose(
                        ep[:, c * 128 : (c + 1) * 128],
                        pacc[:, c * 128 : (c + 1) * 128],
                        ident,
                    )
                nc.vector.tensor_reduce(
                    out=m1t[:, m1col : m1col + n_ch],
                    in_=ep[:].rearrange("p (c k) -> p c k", k=128),
                    axis=mybir.AxisListType.X,
                    op=Alu.min,
                )
                m1col += n_ch

        nc.sync.dma_start(O1[:], m1t[:])
        nc.sync.dma_start(O2[:], m2t[:])

    # Bacc legalization: splits >1-wait instructions into EventSemaphore
    # chains (TRN2 allows 1 wait/inst), moves matmul waits, fuses nops.
    nc.compile()
    return nc


def _split3(x):
    """fp32 -> three bf16 arrays with x ~= b0+b1+b2 (error ~2^-27 |x|)."""
    import ml_dtypes

    bf = ml_dtypes.bfloat16
    b0 = x.astype(bf)
    r1 = (x - b0.astype(np.float32)).astype(np.float32)
    b1 = r1.astype(bf)
    r2 = (r1 - b1.astype(np.float32)).astype(np.float32)
    b2 = r2.astype(bf)
    return b0, b1, b2


# product-pair pattern per coordinate: (gt split idx, pred split idx)
_PAIRS = ((0, 0), (0, 1), (1, 0), (0, 2), (1, 1), (2, 0))


def pack_inputs(pred_filtered, gt_filtered, pred_nonfiltered, gt_nonfiltered):
    """Build per-core input maps (bf16 split operands)."""
    import ml_dtypes

    bf = ml_dtypes.bfloat16

    def mk(p, q):
        # returns (P [B,24,Np] , G [B,24,Nq]) bf16
        p = p.astype(np.float32)
        q = q.astype(np.float32)
        Bn, Np_, _ = p.shape
        Nq = q.shape[1]
        P = np.zeros((Bn, K24, Np_), bf)
        G = np.zeros((Bn, K24, Nq), bf)
        pp = np.sum(p * p, axis=-1, dtype=np.float32)
        qq = np.sum(q * q, axis=-1, dtype=np.float32)
        for c in range(3):
            ws = _split3(-2.0 * p[..., c])     # pred-side coord splits
            gs = _split3(q[..., c])            # gt-side coord splits
            for t, (gi, wi) in enumerate(_PAIRS):
                G[:, 6 * c + t, :] = gs[gi]
                P[:, 6 * c + t, :] = ws[wi]
        qqs = _split3(qq)
        pps = _split3(pp)
        for t in range(3):
            G[:, 18 + t, :] = qqs[t]
            P[:, 18 + t, :] = np.ones_like(pp, dtype=bf)
            G[:, 21 + t, :] = np.ones_like(qq, dtype=bf)
            P[:, 21 + t, :] = pps[t]
        return P, G

    pf_all, gf = mk(pred_filtered, gt_filtered)
    pn_all, gn = mk(pred_nonfiltered, gt_nonfiltered)
    gf = np.ascontiguousarray(gf)
    gn = np.ascontiguousarray(gn)

    in_maps = []
    for k in range(NCORES):
        in_maps.append(
            {
                "pf": np.ascontiguousarray(pf_all[:, :, k * RF : (k + 1) * RF]),
                "gf": gf,
                "pn": np.ascontiguousarray(pn_all[:, :, k * RN : (k + 1) * RN]),
                "gn": gn,
            }
        )
    return in_maps


def combine_outputs(results):
    """results: list (per core) of {"m1": [128,48], "m2": [128,384]} -> loss."""
    cds = {}
    for cfg, (Npts, Rrows, m1off, m2off) in (
        ("f", (NF, RF, 0, 0)),
        ("n", (NN, RN, B * (RF // 128), B * (NF // 128))),
    ):
        n_ch = Rrows // 128   # m1 cols per batch (pred rows / 128, per core)
        n_jt = Npts // 128    # m2 cols per batch (gt chunks)
        # pred-side: values are per-pred-row mins already; mean over all
        m1 = np.stack(
            [r["m1"][:, m1off : m1off + B * n_ch] for r in results], axis=0
        ).reshape(NCORES, 128, B, n_ch)
        pred_mean = m1.mean(axis=(0, 1, 3))  # [B]
        # gt-side: per-core partial mins -> min across cores, mean over gt
        m2 = np.stack(
            [r["m2"][:, m2off : m2off + B * n_jt] for r in results], axis=0
        )
        m2 = m2.min(axis=0).reshape(128, B, n_jt)
        gt_mean = m2.mean(axis=(0, 2))  # [B]
        cds[cfg] = (pred_mean + gt_mean).mean()
    return np.float32(0.7 * cds["f"] + 0.3 * cds["n"])


def kernel(pred_filtered, gt_filtered, pred_nonfiltered, gt_nonfiltered):
    from concourse.bass_utils import run_bass_kernel_spmd

    if "nc" not in _CACHE:
        _CACHE["nc"] = build_nc()
    in_maps = pack_inputs(
        pred_filtered, gt_filtered, pred_nonfiltered, gt_nonfiltered
    )
    res = run_bass_kernel_spmd(_CACHE["nc"], in_maps, core_ids=list(range(NCORES)))
    return combine_outputs(res.results)


# revision 24
# speedup vs baseline: 1.1997x; 1.0483x over previous
"""Bidirectional Chamfer loss on 8 Trainium2 NeuronCores.

Math: for each batch pair (p, q):
    D[i, j] = ||p_i||^2 + ||q_j||^2 - 2 p_i . q_j
    cd = mean_i min_j D[i, j] + mean_j min_i D[i, j]
    loss = 0.7 * mean_b cd_filtered + 0.3 * mean_b cd_nonfiltered

Mapping ("orientation B": gt points on PSUM partitions, pred rows on free):
  - Host packs, per (config, batch), K=24 bf16 matmul operands so one PE
    matmul emits D tiles directly into PSUM. fp32 values are split 3-way
    into bf16 (hi, mid, lo; x = x0+x1+x2 with |x_i| <= 2^-9|x_{i-1}|) and
    products keep the 6 dominant cross terms -> error ~2^-27 per term
    (hardware fp32/fp32r matmul paths are NOT full fp32 precision).
        per coord c:   G rows q_c{0,1,2}, P rows (-2 p_c){0,1,2},
                       pairs (0,0),(0,1),(1,0),(0,2),(1,1),(2,0)
        norm rows:     G [qq0,qq1,qq2,1,1,1] x P [1,1,1,pp0,pp1,pp2]
    -> psum[p, f] = D[pred row f, gt pt jt*128+p]
  - pred rows sharded 8 ways (512/1024 rows per core); gt replicated.
  - Per gt chunk jt (standard BIR ops only — custom DVE ops like
    tensor_tensor_reduce crash this runtime):
      ACT scalar.copy downcasts the PSUM tile to a bf16 SBUF copy (parallel
          engine; only ACT reads PSUM)
      DVE folding tree on the bf16 copy (pairwise tensor_tensor(min) @2x,
          then a short 1x reduce) -> per-gt-point min over this core's pred
          rows (host min-combines across cores)
      DVE tensor_tensor(min) bf16 @2x: predacc accumulates pred-side
          partial mins across gt chunks into [128, R_core].
  - Epilogue: PE-transpose predacc in [128,128] chunks, free-axis reduce
    -> pred-side min per pred row (bf16-rounded, unbiased).
  - Host: tiny cross-core min/mean combine.
"""

import numpy as np

B = 4
NF = 4096
NN = 8192
NCORES = 8
RF = NF // NCORES   # 512 pred rows per core (filtered)
RN = NN // NCORES   # 1024 pred rows per core (nonfiltered)
BIG = 3.0e38
K24 = 24            # contraction rows of the split-bf16 matmul

# output column layout
N_M1 = B * (RF // 128 + RN // 128)          # 4*(4+8) = 48   pred-side mins
N_M2 = B * (NF // 128 + NN // 128)          # 4*(32+64) = 384 gt-side mins

_CACHE = {}


def build_nc():
    """Build the per-core Bass program (SPMD: same program, different data)."""
    from contextlib import ExitStack

    import concourse.mybir as mybir
    import concourse.tile as tile
    from concourse import bacc
    from concourse.masks import make_identity

    f32 = mybir.dt.float32
    bf16 = mybir.dt.bfloat16
    Alu = mybir.AluOpType

    nc = bacc.Bacc("TRN2", target_bir_lowering=False, debug=False)

    # pred (sharded), gt (replicated) operands, [B, K24, n]
    Pf = nc.dram_tensor("pf", [B, K24, RF], bf16, kind="ExternalInput").ap()
    Gf = nc.dram_tensor("gf", [B, K24, NF], bf16, kind="ExternalInput").ap()
    Pn = nc.dram_tensor("pn", [B, K24, RN], bf16, kind="ExternalInput").ap()
    Gn = nc.dram_tensor("gn", [B, K24, NN], bf16, kind="ExternalInput").ap()
    O1 = nc.dram_tensor("m1", [128, N_M1], f32, kind="ExternalOutput").ap()
    O2 = nc.dram_tensor("m2", [128, N_M2], f32, kind="ExternalOutput").ap()

    with tile.TileContext(nc) as tc, ExitStack() as ctx:
        const_pool = ctx.enter_context(tc.tile_pool(name="const", bufs=1))
        gpool = ctx.enter_context(tc.tile_pool(name="gt", bufs=2))
        ppool = ctx.enter_context(tc.tile_pool(name="pred", bufs=2))
        cpool = ctx.enter_context(tc.tile_pool(name="copy", bufs=6))
        spool = ctx.enter_context(tc.tile_pool(name="fold", bufs=3))
        stpool = ctx.enter_context(tc.tile_pool(name="stage", bufs=2))
        apool = ctx.enter_context(tc.tile_pool(name="pacc", bufs=2))
        out_pool = ctx.enter_context(tc.tile_pool(name="outs", bufs=1))
        psum_pool = ctx.enter_context(tc.tile_pool(name="psum", bufs=3, space="PSUM"))
        epi_pool = ctx.enter_context(tc.tile_pool(name="epi", bufs=1, space="PSUM"))

        ident = const_pool.tile([128, 128], bf16)
        make_identity(nc, ident)
        m1t = out_pool.tile([128, N_M1], f32)
        m2t = out_pool.tile([128, N_M2], f32)

        m1col = 0
        m2col = 0
        for Pt, Gt, Npts, Rrows in ((Pf, Gf, NF, RF), (Pn, Gn, NN, RN)):
            n_jt = Npts // 128      # gt chunks
            n_mm = Rrows // 512     # matmuls (N=512) per gt chunk
            for b in range(B):
                sG = gpool.tile([K24, Npts], bf16, tag="gt")
                nc.sync.dma_start(sG[:], Gt[b])
                sP = ppool.tile([K24, Rrows], bf16, tag="pred")
                nc.sync.dma_start(sP[:], Pt[b])
                pacc = apool.tile([128, Rrows], bf16, tag="pacc")

                for jt in range(n_jt):
                    lhsT = sG[:, jt * 128 : (jt + 1) * 128]
                    ps = psum_pool.tile([128, Rrows], f32, tag="ps")
                    for h in range(n_mm):
                        nc.tensor.matmul(
                            ps[:, h * 512 : (h + 1) * 512],
                            lhsT=lhsT,
                            rhs=sP[:, h * 512 : (h + 1) * 512],
                            start=True,
                            stop=True,
                        )
                    cp = cpool.tile([128, Rrows], bf16, tag="copy")
                    nc.scalar.copy(cp[:], ps[:])
                    # m2: two fold-min levels (bf16 @2x), stage the residues
                    # of `grp` consecutive gt chunks, one wide reduce per group
                    res_w = Rrows // 4          # residue width after 2 folds
                    grp = 1024 // res_w         # chunks per staged reduce
                    slot = jt % grp
                    if slot == 0:
                        stg = stpool.tile([128, 1024], bf16, tag="stage")
                    sc = spool.tile([128, Rrows // 2], bf16, tag="fold")
                    w = Rrows // 2
                    nc.vector.tensor_tensor(
                        out=sc[:, :w], in0=cp[:, :w], in1=cp[:, w:], op=Alu.min
                    )
                    nc.vector.tensor_tensor(
                        out=stg[:, slot * res_w : (slot + 1) * res_w],
                        in0=sc[:, :res_w],
                        in1=sc[:, res_w:w],
                        op=Alu.min,
                    )
                    if slot == grp - 1:
                        nc.vector.tensor_reduce(
                            out=m2t[:, m2col + jt - slot : m2col + jt + 1],
                            in_=stg[:].rearrange("p (g k) -> p g k", k=res_w),
                            axis=mybir.AxisListType.X,
                            op=Alu.min,
                        )
                    # m1 partial: accumulate pred-side mins across gt chunks
                    if jt == 0:
                        nc.vector.tensor_copy(pacc[:], cp[:])
                    else:
                        nc.vector.tensor_tensor(
                            out=pacc[:], in0=cp[:], in1=pacc[:], op=Alu.min
                        )
                m2col += n_jt

                # epilogue: pred-side mins (partition-min of pacc via transpose)
                n_ch = Rrows // 128
                ep = epi_pool.tile([128, Rrows], bf16, tag="epi")
                for c in range(n_ch):
                    nc.tensor.transpose(
                        ep[:, c * 128 : (c + 1) * 128],
                        pacc[:, c * 128 : (c + 1) * 128],
                        ident,
                    )
                nc.vector.tensor_reduce(
                    out=m1t[:, m1col : m1col + n_ch],
                    in_=ep[:].rearrange("p (c k) -> p c k", k=128),
                    axis=mybir.AxisListType.X,
                    op=Alu.min,
                )
                m1col += n_ch

        nc.sync.dma_start(O1[:], m1t[:])
        nc.sync.dma_start(O2[:], m2t[:])

    # Bacc legalization: splits >1-wait instructions into EventSemaphore
    # chains (TRN2 allows 1 wait/inst), moves matmul waits, fuses nops.
    nc.compile()
    return nc


def _split3(x):
    """fp32 -> three bf16 arrays with x ~= b0+b1+b2 (error ~2^-27 |x|)."""
    import ml_dtypes

    bf = ml_dtypes.bfloat16
    b0 = x.astype(bf)
    r1 = (x - b0.astype(np.float32)).astype(np.float32)
    b1 = r1.astype(bf)
    r2 = (r1 - b1.astype(np.float32)).astype(np.float32)
    b2 = r2.astype(bf)
    return b0, b1, b2


# product-pair pattern per coordinate: (gt split idx, pred split idx)
_PAIRS = ((0, 0), (0, 1), (1, 0), (0, 2), (1, 1), (2, 0))


def pack_inputs(pred_filtered, gt_filtered, pred_nonfiltered, gt_nonfiltered):
    """Build per-core input maps (bf16 split operands)."""
    import ml_dtypes

    bf = ml_dtypes.bfloat16

    def mk(p, q):
        # returns (P [B,24,Np] , G [B,24,Nq]) bf16
        p = p.astype(np.float32)
        q = q.astype(np.float32)
        Bn, Np_, _ = p.shape
        Nq = q.shape[1]
        P = np.zeros((Bn, K24, Np_), bf)
        G = np.zeros((Bn, K24, Nq), bf)
        pp = np.sum(p * p, axis=-1, dtype=np.float32)
        qq = np.sum(q * q, axis=-1, dtype=np.float32)
        for c in range(3):
            ws = _split3(-2.0 * p[..., c])     # pred-side coord splits
            gs = _split3(q[..., c])            # gt-side coord splits
            for t, (gi, wi) in enumerate(_PAIRS):
                G[:, 6 * c + t, :] = gs[gi]
                P[:, 6 * c + t, :] = ws[wi]
        qqs = _split3(qq)
        pps = _split3(pp)
        for t in range(3):
            G[:, 18 + t, :] = qqs[t]
            P[:, 18 + t, :] = np.ones_like(pp, dtype=bf)
            G[:, 21 + t, :] = np.ones_like(qq, dtype=bf)
            P[:, 21 + t, :] = pps[t]
        return P, G

    pf_all, gf = mk(pred_filtered, gt_filtered)
    pn_all, gn = mk(pred_nonfiltered, gt_nonfiltered)
    gf = np.ascontiguousarray(gf)
    gn = np.ascontiguousarray(gn)

    in_maps = []
    for k in range(NCORES):
        in_maps.append(
            {
                "pf": np.ascontiguousarray(pf_all[:, :, k * RF : (k + 1) * RF]),
                "gf": gf,
                "pn": np.ascontiguousarray(pn_all[:, :, k * RN : (k + 1) * RN]),
                "gn": gn,
            }
        )
    return in_maps


def combine_outputs(results):
    """results: list (per core) of {"m1": [128,48], "m2": [128,384]} -> loss."""
    cds = {}
    for cfg, (Npts, Rrows, m1off, m2off) in (
        ("f", (NF, RF, 0, 0)),
        ("n", (NN, RN, B * (RF // 128), B * (NF // 128))),
    ):
        n_ch = Rrows // 128   # m1 cols per batch (pred rows / 128, per core)
        n_jt = Npts // 128    # m2 cols per batch (gt chunks)
        # pred-side: values are per-pred-row mins already; mean over all
        m1 = np.stack(
            [r["m1"][:, m1off : m1off + B * n_ch] for r in results], axis=0
        ).reshape(NCORES, 128, B, n_ch)
        pred_mean = m1.mean(axis=(0, 1, 3))  # [B]
        # gt-side: per-core partial mins -> min across cores, mean over gt
        m2 = np.stack(
            [r["m2"][:, m2off : m2off + B * n_jt] for r in results], axis=0
        )
        m2 = m2.min(axis=0).reshape(128, B, n_jt)
        gt_mean = m2.mean(axis=(0, 2))  # [B]
        cds[cfg] = (pred_mean + gt_mean).mean()
    return np.float32(0.7 * cds["f"] + 0.3 * cds["n"])


def kernel(pred_filtered, gt_filtered, pred_nonfiltered, gt_nonfiltered):
    from concourse.bass_utils import run_bass_kernel_spmd

    if "nc" not in _CACHE:
        _CACHE["nc"] = build_nc()
    in_maps = pack_inputs(
        pred_filtered, gt_filtered, pred_nonfiltered, gt_nonfiltered
    )
    res = run_bass_kernel_spmd(_CACHE["nc"], in_maps, core_ids=list(range(NCORES)))
    return combine_outputs(res.results)
